# revision 36
# baseline (speedup 1.0000x reference)
"""Trainium2 Bass kernel for nn_Attention_29635274342682 (sparse_attention).

Reference semantics: per-modality (MoE) QKV projection -> per-head RMS-norm
(weight zeros -> scale 1) -> RoPE -> block-diagonal attention over 8 chunks
of 1024 tokens (GQA 24q/8kv heads, hd=128) -> per-modality output projection.
Biases / norm weights are zeros by construction (spec fill "zeros"), so they
are not device inputs.

Sharding: context parallel, core i <- token chunk i (1024 tokens).  Chunk
boundaries coincide with both the attention ranges (CHUNK=1024) and the
modality split (4 chunks per modality), so there is NO cross-core
communication: each core runs the full pipeline on its chunk with its
modality's weights.

Host-side marshalling (in kernel(), pure layout work, no FLOPs): inputs are
sliced per core, cast to bf16 (matmul compute dtype; fp32 accumulation on
device) and pre-transposed so every matmul operand arrives contraction-on-
partitions via plain strided DMA loads.  cos/sin are pre-duplicated to the
rotate-half layout.

Device pipeline per core:
  1. qkv[t,o] = xT.T @ wqT       (PSUM fp32, o-tiles of 512 = 4 heads)
  2. q/k: RMS norm over head dim + RoPE, batched 2 heads per DVE op in the
     bf16 4x mode; the 1/HD mean factor of the RMS norm is folded into the
     softmax exp scale.  bf16 staging is transposed to qT/kT [hd, t] on the
     PE (identity transpose), software-pipelined one psum-tile behind.
  3. scoresT[kt, qt] = kT.T @ qT; P = exp(s*scale - sqrt(HD)) on ACT
     (shift is softmax-invariant; Cauchy-Schwarz bounds |s| <= sqrt(HD)).
  4. oT_unnorm[hd, q] = v.T-chunks @ P-chunks (N=512); the softmax
     denominator is computed partition-broadcast by all-ones stationary
     matmuls over pairwise k-chunk sums of P (DVE add-tree splits the
     reduction between DVE and PE); DVE reciprocal + multiply -> oT bf16.
  5. out[t, ho] = oT.T @ woT -> fp32 -> DRAM.

Measured on HW (trn2, 8 cores): ~0.99 ms NEFF exec, rel err ~6.8e-3 vs the
fp32 reference.  The PE runs saturated back-to-back N=512 bf16 matmuls
(~216 ns issue rate) through all three phases.
"""

import os
import sys

import numpy as np

if os.path.isdir("/opt/trn_rl_repo") and "/opt/trn_rl_repo" not in sys.path:
    sys.path.insert(0, "/opt/trn_rl_repo")

S = 8192
HID = 3072
NHQ = 24
NHKV = 8
GQ = NHQ // NHKV  # 3
HD = 128
HH = HD // 2
NM = 2
CH = 1024  # tokens per core == attention chunk
QKV_OUT = (NHQ + 2 * NHKV) * HD  # 5120
EPS = 1e-6
NCORES = 8
TT = CH // 128  # 8 token tiles per core
KC = HID // 128  # 24 contraction chunks

ESCALE = float(HD) ** 0.5
ESHIFT = -(float(HD) ** 0.5)

OT = 512  # qkv projection o-tile (4 heads)
HOT = 256  # out projection ho-tile


def _build_graph():
    import concourse.mybir as mybir
    import concourse.tile as tile
    from concourse import bacc

    f32 = mybir.dt.float32
    bf16 = mybir.dt.bfloat16
    AF = mybir.ActivationFunctionType

    nc = bacc.Bacc(None, target_bir_lowering=False)

    xT_d = nc.declare_dram_parameter("xT", [HID, CH], bf16, isOutput=False)
    wqT_d = nc.declare_dram_parameter("wqT", [HID, QKV_OUT], bf16, isOutput=False)
    woT_d = nc.declare_dram_parameter("woT", [HID, HID], bf16, isOutput=False)
    ctt_d = nc.declare_dram_parameter("ctt", [CH, HD], bf16, isOutput=False)
    stt_d = nc.declare_dram_parameter("stt", [CH, HD], bf16, isOutput=False)
    out_d = nc.declare_dram_parameter("out", [CH, HID], f32, isOutput=True)

    with tile.TileContext(nc) as tc:
        with nc.allow_low_precision(reason="bf16 staging for matmul operands"):
            _body(tc, mybir, f32, bf16, AF, xT_d, wqT_d, woT_d, ctt_d, stt_d, out_d)
    nc.finalize()
    return nc


class _Ctx:
    pass


def _body(tc, mybir, f32, bf16, AF, xT_d, wqT_d, woT_d, ctt_d, stt_d, out_d):
    from concourse.masks import make_identity

    nc = tc.nc
    c = _Ctx()
    c.nc = nc
    c.mybir = mybir
    c.f32, c.bf16, c.AF = f32, bf16, AF

    with tc.tile_pool(name="consts", bufs=1) as consts:
        c.bias_eps = consts.tile([128, 1], f32)
        nc.vector.memset(c.bias_eps[:], float(HD) * EPS)
        c.bias_shift = consts.tile([128, 1], f32)
        nc.vector.memset(c.bias_shift[:], ESHIFT)
        c.ident = consts.tile([128, 128], bf16)
        make_identity(nc, c.ident[:])
        c.ones = consts.tile([128, 128], bf16)
        nc.vector.memset(c.ones[:], 1.0)

        qkvp = tc.alloc_tile_pool(name="qkvp", bufs=1)
        c.ctt = qkvp.tile([128, TT, HD], bf16)
        c.stt = qkvp.tile([128, TT, HD], bf16)
        nc.sync.dma_start(c.ctt[:], ctt_d.rearrange("(a p) d -> p a d", p=128))
        nc.sync.dma_start(c.stt[:], stt_d.rearrange("(a p) d -> p a d", p=128))

        c.qT = qkvp.tile([128, NHQ, CH], bf16)
        c.kT = qkvp.tile([128, NHKV, CH], bf16)
        c.v = qkvp.tile([128, NHKV * TT, HD], bf16)

        _phase_qkv(tc, c, xT_d, wqT_d)

        oT_pool = tc.alloc_tile_pool(name="oTp", bufs=1, side="right")
        c.oTT = oT_pool.tile([128, NHQ, CH], bf16)
        _phase_attention(tc, c, woT_d, out_d)
        qkvp.release()
        oT_pool.release()


def _phase_qkv(tc, c, xT_d, wqT_d):
    nc = c.nc
    f32, bf16 = c.f32, c.bf16

    with (
        tc.tile_pool(name="xT", bufs=1) as xTp,
        tc.tile_pool(name="wt", bufs=2) as wtp,
        tc.tile_pool(name="psA", bufs=5, space="PSUM") as psA,
        tc.tile_pool(name="psT", bufs=3, space="PSUM") as psTp,
        tc.tile_pool(name="scr", bufs=3) as scr,
        tc.tile_pool(name="stats", bufs=6) as stats,
        tc.tile_pool(name="qstg", bufs=4) as qstgp,
    ):
        xTv = xT_d.rearrange("(k p) t -> p k t", p=128)
        xTall = xTp.tile([128, KC, CH], bf16)
        wqv = wqT_d.rearrange("(k p) o -> p k o", p=128)

        def load_wt(ot, nsplit=1):
            wt = wtp.tile([128, KC, OT], bf16, tag="wt", name="wt")
            step = KC // nsplit
            for s in range(nsplit):
                nc.sync.dma_start(
                    wt[:, s * step : (s + 1) * step, :],
                    wqv[:, s * step : (s + 1) * step, ot * OT : (ot + 1) * OT],
                )
            return wt

        # truly interleave xT chunk loads with the first wt group's per-chunk
        # sub-loads so the k=0 operands of both sides arrive first
        wt_next = wtp.tile([128, KC, OT], bf16, tag="wt", name="wt0")
        for k in range(KC):
            nc.sync.dma_start(wt_next[:, k, :], wqv[:, k, 0:OT])
            nc.sync.dma_start(xTall[:, k, :], xTv[:, k, :])

        pending = []  # deferred PE transposes (1 psum-tile deep pipeline)

        def flush_pending():
            while pending:
                pending.pop(0)()

        n_ot = QKV_OUT // OT  # 10
        for ot in range(n_ot):
            o0 = ot * OT
            wt = wt_next
            if ot + 1 < n_ot:
                wt_next = load_wt(ot + 1, nsplit=4)
            for t in range(TT):
                ps = psA.tile([128, OT], f32, tag="psA", name="psA")
                for k in range(KC):
                    nc.tensor.matmul(
                        ps[:],
                        lhsT=xTall[:, k, t * 128 : (t + 1) * 128],
                        rhs=wt[:, k, :],
                        start=(k == 0),
                        stop=(k == KC - 1),
                    )
                flush_pending()
                for half in range(OT // 256):
                    _evict_qkv_pair(
                        c, ps[:, half * 256 : (half + 1) * 256], o0 + half * 256,
                        t, scr, stats, qstgp, psTp, pending,
                    )
        flush_pending()


def _evict_qkv_pair(c, ps, o0, t, scr, stats, qstgp, psTp, pending):
    """Consume a [128, 256] fp32 qkv PSUM slice (2 heads)."""
    nc = c.nc
    f32, bf16, AF = c.f32, c.bf16, c.AF

    if o0 >= (NHQ + NHKV) * HD:  # v region: plain bf16 cast, natural layout
        for j in range(2):
            vh = (o0 - (NHQ + NHKV) * HD) // HD + j
            nc.scalar.copy(c.v[:, vh * TT + t, :], ps[:, j * HD : (j + 1) * HD])
        return

    if o0 < NHQ * HD:
        dstT, h0 = c.qT, o0 // HD
    else:
        dstT, h0 = c.kT, (o0 - NHQ * HD) // HD

    # RMS stats: per-head sum of squares via ACT accumulate
    sq = scr.tile([128, HD], f32, tag="sq", name="sq")
    ssq2 = stats.tile([128, 2], f32, tag="ssq", name="ssq2")
    for j in range(2):
        nc.scalar.activation(
            sq[:], ps[:, j * HD : (j + 1) * HD], AF.Square,
            accum_out=ssq2[:, j : j + 1],
        )
    rt2 = stats.tile([128, 2], f32, tag="rt", name="rt2")
    nc.scalar.activation(rt2[:], ssq2[:], AF.Sqrt, bias=c.bias_eps[:], scale=1.0)
    rr2 = stats.tile([128, 2], f32, tag="rr", name="rr2")
    nc.vector.reciprocal(rr2[:], rt2[:])

    # qn = q / rms in (half, head, d) permuted bf16 layout: RoPE ops below are
    # contiguous 2D [128, 128] covering both heads in the DVE 4x bf16 mode
    qn = scr.tile([128, 256], bf16, tag="qn", name="qn")
    nc.vector.tensor_mul(
        qn.rearrange("p (f h d) -> p f h d", f=2, h=2),
        ps.rearrange("p (h f d) -> p f h d", h=2, f=2),
        rr2.rearrange("p h -> p () h ()").to_broadcast((128, 2, 2, HH)),
    )

    ct = c.ctt[:, t, :]  # [ct | ct] matches the (h0, h1) lo/hi block layout
    st = c.stt[:, t, :]
    qs = qstgp.tile([128, 256], bf16, tag="qs", name="qs")
    qs_h = qs.rearrange("p (h f d) -> p h f d", h=2, f=2)
    t0 = scr.tile([128, HD], bf16, tag="t0", name="t0")
    t1 = scr.tile([128, HD], bf16, tag="t1", name="t1")
    nc.vector.tensor_mul(t0[:], qn[:, 0:HD], ct)
    nc.vector.tensor_mul(t1[:], qn[:, HD:256], st)
    nc.vector.tensor_sub(
        qs_h[:, :, 0, :],
        t0.rearrange("p (h d) -> p h d", h=2),
        t1.rearrange("p (h d) -> p h d", h=2),
    )
    t2 = scr.tile([128, HD], bf16, tag="t0", name="t2")
    t3 = scr.tile([128, HD], bf16, tag="t1", name="t3")
    nc.vector.tensor_mul(t2[:], qn[:, HD:256], ct)
    nc.vector.tensor_mul(t3[:], qn[:, 0:HD], st)
    nc.vector.tensor_add(
        qs_h[:, :, 1, :],
        t2.rearrange("p (h d) -> p h d", h=2),
        t3.rearrange("p (h d) -> p h d", h=2),
    )

    is_q = o0 < NHQ * HD

    def emit_transposes(qs=qs, dstT=dstT, h0=h0, t=t, is_q=is_q):
        for j in range(2):
            pst = psTp.tile([128, 128], bf16, tag="psT", name="psT")
            nc.tensor.transpose(pst[:], qs[:, j * HD : (j + 1) * HD], c.ident[:])
            # split the PSUM->SBUF evictions between DVE and ACT to balance
            if (t + j) % 2 == 0:
                nc.vector.tensor_copy(dstT[:, h0 + j, t * 128 : (t + 1) * 128], pst[:])
            else:
                nc.scalar.copy(dstT[:, h0 + j, t * 128 : (t + 1) * 128], pst[:])

    pending.append(emit_transposes)


def _phase_attention(tc, c, woT_d, out_d):
    """Attention (qc-major) with the out-projection for the first token half
    interleaved into the second attention half: attention is ACT(exp)-bound
    with PE slack, the out-projection is pure PE -- merging overlaps them."""
    nc = c.nc
    f32, bf16, AF = c.f32, c.bf16, c.AF
    QC = 512
    NQC = CH // QC  # 2
    N_HO = HID // HOT

    with (
        tc.tile_pool(name="Pp", bufs=2) as Pp,
        tc.tile_pool(name="psS", bufs=1, space="PSUM") as psS,
        tc.tile_pool(name="psAV", bufs=2, space="PSUM") as psAV,
        tc.tile_pool(name="psD", bufs=2, space="PSUM") as psD,
        tc.tile_pool(name="psB", bufs=2, space="PSUM") as psB,
        tc.tile_pool(name="rsb", bufs=3) as rsbp,
        tc.tile_pool(name="ptree", bufs=2) as ptree,
        tc.tile_pool(name="wt2", bufs=2) as wtp,
        tc.tile_pool(name="outs", bufs=3) as outs,
    ):
        wov = woT_d.rearrange("(k p) o -> p k o", p=128)
        wt2 = {}

        def load_wt2(ho):
            if ho < N_HO and ho not in wt2:
                w = wtp.tile([128, KC, HOT], bf16, tag="wt2", name="wt2")
                nc.sync.dma_start(w[:], wov[:, :, ho * HOT : (ho + 1) * HOT])
                wt2[ho] = w
            return wt2.get(ho)

        def oproj_tile(ho, t):
            w = wt2[ho]
            ps = psB.tile([128, HOT], f32, tag="psB", name="psB")
            for k in range(KC):
                nc.tensor.matmul(
                    ps[:],
                    lhsT=c.oTT[:, k, t * 128 : (t + 1) * 128],
                    rhs=w[:, k, :],
                    start=(k == 0),
                    stop=(k == KC - 1),
                )
            ob = outs.tile([128, HOT], f32, tag="outs", name="ob")
            nc.scalar.copy(ob[:], ps[:])
            nc.gpsimd.dma_start(
                out_d[t * 128 : (t + 1) * 128, ho * HOT : (ho + 1) * HOT], ob[:]
            )

        def emit_item(cur, prev):
            h, qc = cur
            g = h // GQ
            Pt = Pp.tile([128, TT, QC], bf16, tag="P", name="Pt")
            s = []
            if prev is not None:
                ph, pqc, pPt, ps_tree = prev
                pg = ph // GQ
                pav = psAV.tile([128, QC], f32, tag="psAV", name="pav")
            for kc2 in range(TT // 2):
                pss = psS.tile([128, 2 * QC], f32, tag="psS", name="psS")
                for j in range(2):
                    nc.tensor.matmul(
                        pss[:, j * QC : (j + 1) * QC],
                        lhsT=c.kT[:, g, (2 * kc2 + j) * 128 : (2 * kc2 + j + 1) * 128],
                        rhs=c.qT[:, h, qc * QC : (qc + 1) * QC],
                        start=True,
                        stop=True,
                    )
                if prev is not None:
                    for j in range(2):
                        kc = 2 * kc2 + j
                        nc.tensor.matmul(
                            pav[:],
                            lhsT=c.v[:, pg * TT + kc, :],
                            rhs=pPt[:, kc, :],
                            start=(kc == 0),
                            stop=(kc == TT - 1),
                        )
                nc.scalar.activation(
                    Pt.rearrange("p a b -> p (a b)")[
                        :, 2 * kc2 * QC : 2 * (kc2 + 1) * QC
                    ],
                    pss[:], AF.Exp, bias=c.bias_shift[:], scale=ESCALE,
                )
                sj = ptree.tile([128, QC], bf16, tag=f"pt{kc2}", name=f"s{kc2}")
                nc.vector.tensor_add(
                    sj[:], Pt[:, 2 * kc2, :], Pt[:, 2 * kc2 + 1, :]
                )
                s.append(sj)
            if prev is not None:
                pd = psD.tile([128, QC], f32, tag="psD", name="pd")
                for j in range(4):
                    nc.tensor.matmul(
                        pd[:], lhsT=c.ones[:], rhs=ps_tree[j][:],
                        start=(j == 0), stop=(j == 3),
                    )
                rsb = rsbp.tile([128, QC], f32, tag="rsb", name="rsb")
                nc.vector.reciprocal(rsb[:], pd[:])
                nc.vector.tensor_mul(
                    c.oTT[:, ph, pqc * QC : (pqc + 1) * QC], pav[:], rsb[:]
                )
            return Pt, s

        def emit_tail(prev):
            ph, pqc, pPt, ps_tree = prev
            pg = ph // GQ
            pav = psAV.tile([128, QC], f32, tag="psAV", name="pav")
            for kc in range(TT):
                nc.tensor.matmul(
                    pav[:],
                    lhsT=c.v[:, pg * TT + kc, :],
                    rhs=pPt[:, kc, :],
                    start=(kc == 0),
                    stop=(kc == TT - 1),
                )
            pd = psD.tile([128, QC], f32, tag="psD", name="pd")
            for j in range(4):
                nc.tensor.matmul(
                    pd[:], lhsT=c.ones[:], rhs=ps_tree[j][:],
                    start=(j == 0), stop=(j == 3),
                )
            rsb = rsbp.tile([128, QC], f32, tag="rsb", name="rsb")
            nc.vector.reciprocal(rsb[:], pd[:])
            nc.vector.tensor_mul(
                c.oTT[:, ph, pqc * QC : (pqc + 1) * QC], pav[:], rsb[:]
            )

        # qc-major: all heads for the first token half, then the second;
        # oproj tiles for t<TT/2 interleave into the second half.
        work = [(h, qc) for qc in range(NQC) for h in range(NHQ)]
        # (ho, t) pairs for the first token half, ho-major for wt2 reuse
        opq = [(ho, t) for ho in range(N_HO) for t in range(TT // 2)]
        first_interleave = NHQ + 1  # first oTT half fully written by then
        n_items = len(work)
        per_item = -(-len(opq) // (n_items - first_interleave))  # ceil
        load_wt2(0)
        opi = 0
        prev = None
        for idx, cur in enumerate(work):
            Pt, s = emit_item(cur, prev)
            prev = (cur[0], cur[1], Pt, s)
            if idx >= first_interleave:
                for _ in range(per_item):
                    if opi < len(opq):
                        ho, t = opq[opi]
                        load_wt2(ho + 1)
                        oproj_tile(ho, t)
                        opi += 1
        emit_tail(prev)
        while opi < len(opq):
            ho, t = opq[opi]
            load_wt2(ho + 1)
            oproj_tile(ho, t)
            opi += 1
        # remaining token half: reload wt2 fresh (old slots were recycled
        # during the interleave)
        wt2.clear()
        load_wt2(0)
        for ho in range(N_HO):
            load_wt2(ho + 1)
            for t in range(TT // 2, TT):
                oproj_tile(ho, t)


_NC_CACHE = None


def _get_nc():
    global _NC_CACHE
    if _NC_CACHE is None:
        _NC_CACHE = _build_graph()
    return _NC_CACHE


def kernel(**inputs) -> np.ndarray:
    import ml_dtypes

    from concourse.bass_utils import run_bass_kernel_spmd

    bf16 = ml_dtypes.bfloat16
    x = np.asarray(inputs["x"], dtype=np.float32)
    w_qkv = np.asarray(inputs["w_qkv"], dtype=np.float32)
    w_out = np.asarray(inputs["w_out"], dtype=np.float32)
    cos = np.asarray(inputs["cos"], dtype=np.float32)
    sin = np.asarray(inputs["sin"], dtype=np.float32)

    # host-side marshalling: per-modality weight transposes (shared by the 4
    # cores of each modality), bf16 compute dtype, rotate-half cos/sin layout
    wqT = [np.ascontiguousarray(w_qkv[m].T).astype(bf16) for m in range(NM)]
    woT = [np.ascontiguousarray(w_out[m].T).astype(bf16) for m in range(NM)]

    in_maps = []
    for i in range(NCORES):
        m = i * NM // NCORES  # cores 0-3 -> modality 0, 4-7 -> modality 1
        sl = slice(i * CH, (i + 1) * CH)
        ctt = np.concatenate([cos[sl], cos[sl]], axis=1).astype(bf16)
        stt = np.concatenate([sin[sl], sin[sl]], axis=1).astype(bf16)
        in_maps.append(
            {
                "xT": np.ascontiguousarray(x[sl].T).astype(bf16),
                "wqT": wqT[m],
                "woT": woT[m],
                "ctt": ctt,
                "stt": stt,
            }
        )

    nc = _get_nc()
    res = run_bass_kernel_spmd(nc, in_maps, core_ids=list(range(NCORES)))
    outs = [np.asarray(res.results[i]["out"]) for i in range(NCORES)]
    return np.concatenate(outs, axis=0).astype(np.float32)


# revision 37
# speedup vs baseline: 1.1114x; 1.1114x over previous
"""Trainium2 Bass kernel for nn_Attention_29635274342682 (sparse_attention).

Reference semantics: per-modality (MoE) QKV projection -> per-head RMS-norm
(weight zeros -> scale 1) -> RoPE -> block-diagonal attention over 8 chunks
of 1024 tokens (GQA 24q/8kv heads, hd=128) -> per-modality output projection.
Biases / norm weights are zeros by construction (spec fill "zeros"), so they
are not device inputs.

Sharding: context parallel, core i <- token chunk i (1024 tokens).  Chunk
boundaries coincide with both the attention ranges (CHUNK=1024) and the
modality split (4 chunks per modality), so there is NO cross-core
communication: each core runs the full pipeline on its chunk with its
modality's weights.

Host-side marshalling (in kernel(), pure layout work, no FLOPs): inputs are
sliced per core, cast to bf16 (matmul compute dtype; fp32 accumulation on
device) and pre-transposed so every matmul operand arrives contraction-on-
partitions via plain strided DMA loads.  cos/sin are pre-duplicated to the
rotate-half layout.

Device pipeline per core:
  1. qkv[t,o] = xT.T @ wqT       (PSUM fp32, o-tiles of 512 = 4 heads)
  2. q/k: RMS norm over head dim + RoPE, batched 2 heads per DVE op in the
     bf16 4x mode; the 1/HD mean factor of the RMS norm is folded into the
     softmax exp scale.  bf16 staging is transposed to qT/kT [hd, t] on the
     PE (identity transpose), software-pipelined one psum-tile behind.
  3. scoresT[kt, qt] = kT.T @ qT; P = exp(s*scale - sqrt(HD)) on ACT
     (shift is softmax-invariant; Cauchy-Schwarz bounds |s| <= sqrt(HD)).
  4. oT_unnorm[hd, q] = v.T-chunks @ P-chunks (N=512); the softmax
     denominator is computed partition-broadcast by all-ones stationary
     matmuls over pairwise k-chunk sums of P (DVE add-tree splits the
     reduction between DVE and PE); DVE reciprocal + multiply -> oT bf16.
  5. out[t, ho] = oT.T @ woT -> fp32 -> DRAM.

Measured on HW (trn2, 8 cores): ~0.99 ms NEFF exec, rel err ~6.8e-3 vs the
fp32 reference.  The PE runs saturated back-to-back N=512 bf16 matmuls
(~216 ns issue rate) through all three phases.
"""

import os
import sys

import numpy as np

if os.path.isdir("/opt/trn_rl_repo") and "/opt/trn_rl_repo" not in sys.path:
    sys.path.insert(0, "/opt/trn_rl_repo")

S = 8192
HID = 3072
NHQ = 24
NHKV = 8
GQ = NHQ // NHKV  # 3
HD = 128
HH = HD // 2
NM = 2
CH = 1024  # tokens per core == attention chunk
QKV_OUT = (NHQ + 2 * NHKV) * HD  # 5120
EPS = 1e-6
NCORES = 8
TT = CH // 128  # 8 token tiles per core
KC = HID // 128  # 24 contraction chunks

ESCALE = float(HD) ** 0.5
ESHIFT = -(float(HD) ** 0.5)

OT = 512  # qkv projection o-tile (4 heads)
HOT = 256  # out projection ho-tile


def _build_graph():
    import concourse.mybir as mybir
    import concourse.tile as tile
    from concourse import bacc

    f32 = mybir.dt.float32
    bf16 = mybir.dt.bfloat16
    AF = mybir.ActivationFunctionType

    nc = bacc.Bacc(None, target_bir_lowering=False)

    xT_d = nc.declare_dram_parameter("xT", [HID, CH], bf16, isOutput=False)
    wqT_d = nc.declare_dram_parameter("wqT", [HID, QKV_OUT], bf16, isOutput=False)
    woT_d = nc.declare_dram_parameter("woT", [HID, HID], bf16, isOutput=False)
    ctt_d = nc.declare_dram_parameter("ctt", [CH, HD], bf16, isOutput=False)
    stt_d = nc.declare_dram_parameter("stt", [CH, HD], bf16, isOutput=False)
    out_d = nc.declare_dram_parameter("out", [CH, HID], f32, isOutput=True)

    with tile.TileContext(nc) as tc:
        with nc.allow_low_precision(reason="bf16 staging for matmul operands"):
            _body(tc, mybir, f32, bf16, AF, xT_d, wqT_d, woT_d, ctt_d, stt_d, out_d)
    nc.finalize()
    return nc


class _Ctx:
    pass


def _body(tc, mybir, f32, bf16, AF, xT_d, wqT_d, woT_d, ctt_d, stt_d, out_d):
    from concourse.masks import make_identity

    nc = tc.nc
    c = _Ctx()
    c.nc = nc
    c.mybir = mybir
    c.f32, c.bf16, c.AF = f32, bf16, AF

    with tc.tile_pool(name="consts", bufs=1) as consts:
        c.bias_eps = consts.tile([128, 1], f32)
        nc.vector.memset(c.bias_eps[:], float(HD) * EPS)
        c.bias_shift = consts.tile([128, 1], f32)
        nc.vector.memset(c.bias_shift[:], ESHIFT)
        c.ident = consts.tile([128, 128], bf16)
        make_identity(nc, c.ident[:])
        c.ones = consts.tile([128, 128], bf16)
        nc.vector.memset(c.ones[:], 1.0)

        qkvp = tc.alloc_tile_pool(name="qkvp", bufs=1)
        c.ctt = qkvp.tile([128, TT, HD], bf16)
        c.stt = qkvp.tile([128, TT, HD], bf16)
        nc.sync.dma_start(c.ctt[:], ctt_d.rearrange("(a p) d -> p a d", p=128))
        nc.sync.dma_start(c.stt[:], stt_d.rearrange("(a p) d -> p a d", p=128))

        c.qT = qkvp.tile([128, NHQ, CH], bf16)
        c.kT = qkvp.tile([128, NHKV, CH], bf16)
        c.v = qkvp.tile([128, NHKV * TT, HD], bf16)

        _phase_qkv(tc, c, xT_d, wqT_d)

        oT_pool = tc.alloc_tile_pool(name="oTp", bufs=1, side="right")
        c.oTT = oT_pool.tile([128, NHQ, CH], bf16)
        _phase_attention(tc, c, woT_d, out_d)
        qkvp.release()
        oT_pool.release()


def _phase_qkv(tc, c, xT_d, wqT_d):
    nc = c.nc
    f32, bf16 = c.f32, c.bf16

    with (
        tc.tile_pool(name="xT", bufs=1) as xTp,
        tc.tile_pool(name="wt", bufs=2) as wtp,
        tc.tile_pool(name="psA", bufs=5, space="PSUM") as psA,
        tc.tile_pool(name="psT", bufs=3, space="PSUM") as psTp,
        tc.tile_pool(name="scr", bufs=3) as scr,
        tc.tile_pool(name="stats", bufs=6) as stats,
        tc.tile_pool(name="qstg", bufs=4) as qstgp,
    ):
        xTv = xT_d.rearrange("(k p) t -> p k t", p=128)
        xTall = xTp.tile([128, KC, CH], bf16)
        wqv = wqT_d.rearrange("(k p) o -> p k o", p=128)

        def load_wt(ot, nsplit=1):
            wt = wtp.tile([128, KC, OT], bf16, tag="wt", name="wt")
            step = KC // nsplit
            for s in range(nsplit):
                nc.sync.dma_start(
                    wt[:, s * step : (s + 1) * step, :],
                    wqv[:, s * step : (s + 1) * step, ot * OT : (ot + 1) * OT],
                )
            return wt

        # truly interleave xT chunk loads with the first wt group's per-chunk
        # sub-loads so the k=0 operands of both sides arrive first
        wt_next = wtp.tile([128, KC, OT], bf16, tag="wt", name="wt0")
        for k in range(KC):
            nc.sync.dma_start(wt_next[:, k, :], wqv[:, k, 0:OT])
            nc.sync.dma_start(xTall[:, k, :], xTv[:, k, :])

        pending = []  # deferred PE transposes (1 psum-tile deep pipeline)

        def flush_pending():
            while pending:
                pending.pop(0)()

        n_ot = QKV_OUT // OT  # 10
        for ot in range(n_ot):
            o0 = ot * OT
            wt = wt_next
            if ot + 1 < n_ot:
                wt_next = load_wt(ot + 1, nsplit=4)
            for t in range(TT):
                ps = psA.tile([128, OT], f32, tag="psA", name="psA")
                for k in range(KC):
                    nc.tensor.matmul(
                        ps[:],
                        lhsT=xTall[:, k, t * 128 : (t + 1) * 128],
                        rhs=wt[:, k, :],
                        start=(k == 0),
                        stop=(k == KC - 1),
                    )
                flush_pending()
                for half in range(OT // 256):
                    _evict_qkv_pair(
                        c, ps[:, half * 256 : (half + 1) * 256], o0 + half * 256,
                        t, scr, stats, qstgp, psTp, pending,
                    )
        flush_pending()


def _evict_qkv_pair(c, ps, o0, t, scr, stats, qstgp, psTp, pending):
    """Consume a [128, 256] fp32 qkv PSUM slice (2 heads)."""
    nc = c.nc
    f32, bf16, AF = c.f32, c.bf16, c.AF

    if o0 >= (NHQ + NHKV) * HD:  # v region: plain bf16 cast, natural layout
        for j in range(2):
            vh = (o0 - (NHQ + NHKV) * HD) // HD + j
            nc.scalar.copy(c.v[:, vh * TT + t, :], ps[:, j * HD : (j + 1) * HD])
        return

    if o0 < NHQ * HD:
        dstT, h0 = c.qT, o0 // HD
    else:
        dstT, h0 = c.kT, (o0 - NHQ * HD) // HD

    # RMS stats: per-head sum of squares via ACT accumulate
    sq = scr.tile([128, HD], f32, tag="sq", name="sq")
    ssq2 = stats.tile([128, 2], f32, tag="ssq", name="ssq2")
    for j in range(2):
        nc.scalar.activation(
            sq[:], ps[:, j * HD : (j + 1) * HD], AF.Square,
            accum_out=ssq2[:, j : j + 1],
        )
    rt2 = stats.tile([128, 2], f32, tag="rt", name="rt2")
    nc.scalar.activation(rt2[:], ssq2[:], AF.Sqrt, bias=c.bias_eps[:], scale=1.0)
    rr2 = stats.tile([128, 2], f32, tag="rr", name="rr2")
    nc.vector.reciprocal(rr2[:], rt2[:])

    # qn = q / rms in (half, head, d) permuted bf16 layout: RoPE ops below are
    # contiguous 2D [128, 128] covering both heads in the DVE 4x bf16 mode
    qn = scr.tile([128, 256], bf16, tag="qn", name="qn")
    nc.vector.tensor_mul(
        qn.rearrange("p (f h d) -> p f h d", f=2, h=2),
        ps.rearrange("p (h f d) -> p f h d", h=2, f=2),
        rr2.rearrange("p h -> p () h ()").to_broadcast((128, 2, 2, HH)),
    )

    ct = c.ctt[:, t, :]  # [ct | ct] matches the (h0, h1) lo/hi block layout
    st = c.stt[:, t, :]
    qs = qstgp.tile([128, 256], bf16, tag="qs", name="qs")
    qs_h = qs.rearrange("p (h f d) -> p h f d", h=2, f=2)
    t0 = scr.tile([128, HD], bf16, tag="t0", name="t0")
    t1 = scr.tile([128, HD], bf16, tag="t1", name="t1")
    nc.vector.tensor_mul(t0[:], qn[:, 0:HD], ct)
    nc.vector.tensor_mul(t1[:], qn[:, HD:256], st)
    nc.vector.tensor_sub(
        qs_h[:, :, 0, :],
        t0.rearrange("p (h d) -> p h d", h=2),
        t1.rearrange("p (h d) -> p h d", h=2),
    )
    t2 = scr.tile([128, HD], bf16, tag="t0", name="t2")
    t3 = scr.tile([128, HD], bf16, tag="t1", name="t3")
    nc.vector.tensor_mul(t2[:], qn[:, HD:256], ct)
    nc.vector.tensor_mul(t3[:], qn[:, 0:HD], st)
    nc.vector.tensor_add(
        qs_h[:, :, 1, :],
        t2.rearrange("p (h d) -> p h d", h=2),
        t3.rearrange("p (h d) -> p h d", h=2),
    )

    is_q = o0 < NHQ * HD

    def emit_transposes(qs=qs, dstT=dstT, h0=h0, t=t, is_q=is_q):
        for j in range(2):
            pst = psTp.tile([128, 128], bf16, tag="psT", name="psT")
            nc.tensor.transpose(pst[:], qs[:, j * HD : (j + 1) * HD], c.ident[:])
            # split the PSUM->SBUF evictions between DVE and ACT to balance
            if (t + j) % 2 == 0:
                nc.vector.tensor_copy(dstT[:, h0 + j, t * 128 : (t + 1) * 128], pst[:])
            else:
                nc.scalar.copy(dstT[:, h0 + j, t * 128 : (t + 1) * 128], pst[:])

    pending.append(emit_transposes)


def _phase_attention(tc, c, woT_d, out_d):
    """Attention (qc-major) with the out-projection for the first token half
    interleaved into the second attention half: attention is ACT(exp)-bound
    with PE slack, the out-projection is pure PE -- merging overlaps them."""
    nc = c.nc
    f32, bf16, AF = c.f32, c.bf16, c.AF
    QC = 512
    NQC = CH // QC  # 2
    N_HO = HID // HOT

    with (
        tc.tile_pool(name="Pp", bufs=2) as Pp,
        tc.tile_pool(name="psS", bufs=2, space="PSUM") as psS,
        tc.tile_pool(name="psAV", bufs=2, space="PSUM") as psAV,
        tc.tile_pool(name="psD", bufs=1, space="PSUM") as psD,
        tc.tile_pool(name="psB", bufs=1, space="PSUM") as psB,
        tc.tile_pool(name="rsb", bufs=3) as rsbp,
        tc.tile_pool(name="ptree", bufs=2) as ptree,
        tc.tile_pool(name="wt2", bufs=2) as wtp,
        tc.tile_pool(name="outs", bufs=3) as outs,
    ):
        wov = woT_d.rearrange("(k p) o -> p k o", p=128)
        wt2 = {}

        def load_wt2(ho):
            if ho < N_HO and ho not in wt2:
                w = wtp.tile([128, KC, HOT], bf16, tag="wt2", name="wt2")
                nc.sync.dma_start(w[:], wov[:, :, ho * HOT : (ho + 1) * HOT])
                wt2[ho] = w
            return wt2.get(ho)

        def oproj_tile(ho, t):
            w = wt2[ho]
            ps = psB.tile([128, HOT], f32, tag="psB", name="psB")
            for k in range(KC):
                nc.tensor.matmul(
                    ps[:],
                    lhsT=c.oTT[:, k, t * 128 : (t + 1) * 128],
                    rhs=w[:, k, :],
                    start=(k == 0),
                    stop=(k == KC - 1),
                )
            ob = outs.tile([128, HOT], f32, tag="outs", name="ob")
            nc.scalar.copy(ob[:], ps[:])
            nc.gpsimd.dma_start(
                out_d[t * 128 : (t + 1) * 128, ho * HOT : (ho + 1) * HOT], ob[:]
            )

        def emit_item(cur, prev):
            h, qc = cur
            g = h // GQ
            Pt = Pp.tile([128, TT, QC], bf16, tag="P", name="Pt")
            s = []
            if prev is not None:
                ph, pqc, pPt, ps_tree = prev
                pg = ph // GQ
                pav = psAV.tile([128, QC], f32, tag="psAV", name="pav")
            for kc2 in range(TT // 2):
                pss = psS.tile([128, 2 * QC], f32, tag="psS", name="psS")
                for j in range(2):
                    nc.tensor.matmul(
                        pss[:, j * QC : (j + 1) * QC],
                        lhsT=c.kT[:, g, (2 * kc2 + j) * 128 : (2 * kc2 + j + 1) * 128],
                        rhs=c.qT[:, h, qc * QC : (qc + 1) * QC],
                        start=True,
                        stop=True,
                    )
                if prev is not None:
                    for j in range(2):
                        kc = 2 * kc2 + j
                        nc.tensor.matmul(
                            pav[:],
                            lhsT=c.v[:, pg * TT + kc, :],
                            rhs=pPt[:, kc, :],
                            start=(kc == 0),
                            stop=(kc == TT - 1),
                        )
                nc.scalar.activation(
                    Pt.rearrange("p a b -> p (a b)")[
                        :, 2 * kc2 * QC : 2 * (kc2 + 1) * QC
                    ],
                    pss[:], AF.Exp, bias=c.bias_shift[:], scale=ESCALE,
                )
                sj = ptree.tile([128, QC], bf16, tag=f"pt{kc2}", name=f"s{kc2}")
                nc.vector.tensor_add(
                    sj[:], Pt[:, 2 * kc2, :], Pt[:, 2 * kc2 + 1, :]
                )
                s.append(sj)
            if prev is not None:
                pd = psD.tile([128, QC], f32, tag="psD", name="pd")
                for j in range(4):
                    nc.tensor.matmul(
                        pd[:], lhsT=c.ones[:], rhs=ps_tree[j][:],
                        start=(j == 0), stop=(j == 3),
                    )
                rsb = rsbp.tile([128, QC], f32, tag="rsb", name="rsb")
                nc.vector.reciprocal(rsb[:], pd[:])
                nc.vector.tensor_mul(
                    c.oTT[:, ph, pqc * QC : (pqc + 1) * QC], pav[:], rsb[:]
                )
            return Pt, s

        def emit_tail(prev):
            ph, pqc, pPt, ps_tree = prev
            pg = ph // GQ
            pav = psAV.tile([128, QC], f32, tag="psAV", name="pav")
            for kc in range(TT):
                nc.tensor.matmul(
                    pav[:],
                    lhsT=c.v[:, pg * TT + kc, :],
                    rhs=pPt[:, kc, :],
                    start=(kc == 0),
                    stop=(kc == TT - 1),
                )
            pd = psD.tile([128, QC], f32, tag="psD", name="pd")
            for j in range(4):
                nc.tensor.matmul(
                    pd[:], lhsT=c.ones[:], rhs=ps_tree[j][:],
                    start=(j == 0), stop=(j == 3),
                )
            rsb = rsbp.tile([128, QC], f32, tag="rsb", name="rsb")
            nc.vector.reciprocal(rsb[:], pd[:])
            nc.vector.tensor_mul(
                c.oTT[:, ph, pqc * QC : (pqc + 1) * QC], pav[:], rsb[:]
            )

        # qc-major: all heads for the first token half, then the second;
        # oproj tiles for t<TT/2 interleave into the second half.
        work = [(h, qc) for qc in range(NQC) for h in range(NHQ)]
        # (ho, t) pairs for the first token half, ho-major for wt2 reuse
        opq = [(ho, t) for ho in range(N_HO) for t in range(TT // 2)]
        first_interleave = NHQ + 1  # first oTT half fully written by then
        n_items = len(work)
        per_item = -(-len(opq) // (n_items - first_interleave))  # ceil
        load_wt2(0)
        opi = 0
        prev = None
        for idx, cur in enumerate(work):
            Pt, s = emit_item(cur, prev)
            prev = (cur[0], cur[1], Pt, s)
            if idx >= first_interleave:
                for _ in range(per_item):
                    if opi < len(opq):
                        ho, t = opq[opi]
                        load_wt2(ho + 1)
                        oproj_tile(ho, t)
                        opi += 1
        emit_tail(prev)
        while opi < len(opq):
            ho, t = opq[opi]
            load_wt2(ho + 1)
            oproj_tile(ho, t)
            opi += 1
        # remaining token half: reload wt2 fresh (old slots were recycled
        # during the interleave)
        wt2.clear()
        load_wt2(0)
        for ho in range(N_HO):
            load_wt2(ho + 1)
            for t in range(TT // 2, TT):
                oproj_tile(ho, t)


_NC_CACHE = None


def _get_nc():
    global _NC_CACHE
    if _NC_CACHE is None:
        _NC_CACHE = _build_graph()
    return _NC_CACHE


def kernel(**inputs) -> np.ndarray:
    import ml_dtypes

    from concourse.bass_utils import run_bass_kernel_spmd

    bf16 = ml_dtypes.bfloat16
    x = np.asarray(inputs["x"], dtype=np.float32)
    w_qkv = np.asarray(inputs["w_qkv"], dtype=np.float32)
    w_out = np.asarray(inputs["w_out"], dtype=np.float32)
    cos = np.asarray(inputs["cos"], dtype=np.float32)
    sin = np.asarray(inputs["sin"], dtype=np.float32)

    # host-side marshalling: per-modality weight transposes (shared by the 4
    # cores of each modality), bf16 compute dtype, rotate-half cos/sin layout
    wqT = [np.ascontiguousarray(w_qkv[m].T).astype(bf16) for m in range(NM)]
    woT = [np.ascontiguousarray(w_out[m].T).astype(bf16) for m in range(NM)]

    in_maps = []
    for i in range(NCORES):
        m = i * NM // NCORES  # cores 0-3 -> modality 0, 4-7 -> modality 1
        sl = slice(i * CH, (i + 1) * CH)
        ctt = np.concatenate([cos[sl], cos[sl]], axis=1).astype(bf16)
        stt = np.concatenate([sin[sl], sin[sl]], axis=1).astype(bf16)
        in_maps.append(
            {
                "xT": np.ascontiguousarray(x[sl].T).astype(bf16),
                "wqT": wqT[m],
                "woT": woT[m],
                "ctt": ctt,
                "stt": stt,
            }
        )

    nc = _get_nc()
    res = run_bass_kernel_spmd(nc, in_maps, core_ids=list(range(NCORES)))
    outs = [np.asarray(res.results[i]["out"]) for i in range(NCORES)]
    return np.concatenate(outs, axis=0).astype(np.float32)


# revision 38
# speedup vs baseline: 1.1382x; 1.0241x over previous
"""Trainium2 Bass kernel for nn_Attention_29635274342682 (sparse_attention).

Reference semantics: per-modality (MoE) QKV projection -> per-head RMS-norm
(weight zeros -> scale 1) -> RoPE -> block-diagonal attention over 8 chunks
of 1024 tokens (GQA 24q/8kv heads, hd=128) -> per-modality output projection.
Biases / norm weights are zeros by construction (spec fill "zeros"), so they
are not device inputs.

Sharding: context parallel, core i <- token chunk i (1024 tokens).  Chunk
boundaries coincide with both the attention ranges (CHUNK=1024) and the
modality split (4 chunks per modality), so there is NO cross-core
communication: each core runs the full pipeline on its chunk with its
modality's weights.

Host-side marshalling (in kernel(), pure layout work, no FLOPs): inputs are
sliced per core, cast to bf16 (matmul compute dtype; fp32 accumulation on
device) and pre-transposed so every matmul operand arrives contraction-on-
partitions via plain strided DMA loads.  cos/sin are pre-duplicated to the
rotate-half layout.

Device pipeline per core:
  1. qkv[t,o] = xT.T @ wqT       (PSUM fp32, o-tiles of 512 = 4 heads)
  2. q/k: RMS norm over head dim + RoPE, batched 2 heads per DVE op in the
     bf16 4x mode; the 1/HD mean factor of the RMS norm is folded into the
     softmax exp scale.  bf16 staging is transposed to qT/kT [hd, t] on the
     PE (identity transpose), software-pipelined one psum-tile behind.
  3. scoresT[kt, qt] = kT.T @ qT; P = exp(s*scale - sqrt(HD)) on ACT
     (shift is softmax-invariant; Cauchy-Schwarz bounds |s| <= sqrt(HD)).
  4. oT_unnorm[hd, q] = v.T-chunks @ P-chunks (N=512); the softmax
     denominator is computed partition-broadcast by all-ones stationary
     matmuls over pairwise k-chunk sums of P (DVE add-tree splits the
     reduction between DVE and PE); DVE reciprocal + multiply -> oT bf16.
  5. out[t, ho] = oT.T @ woT -> fp32 -> DRAM.

Measured on HW (trn2, 8 cores): ~0.99 ms NEFF exec, rel err ~6.8e-3 vs the
fp32 reference.  The PE runs saturated back-to-back N=512 bf16 matmuls
(~216 ns issue rate) through all three phases.
"""

import os
import sys

import numpy as np

if os.path.isdir("/opt/trn_rl_repo") and "/opt/trn_rl_repo" not in sys.path:
    sys.path.insert(0, "/opt/trn_rl_repo")

S = 8192
HID = 3072
NHQ = 24
NHKV = 8
GQ = NHQ // NHKV  # 3
HD = 128
HH = HD // 2
NM = 2
CH = 1024  # tokens per core == attention chunk
QKV_OUT = (NHQ + 2 * NHKV) * HD  # 5120
EPS = 1e-6
NCORES = 8
TT = CH // 128  # 8 token tiles per core
KC = HID // 128  # 24 contraction chunks

ESCALE = float(HD) ** 0.5
ESHIFT = -(float(HD) ** 0.5)

OT = 512  # qkv projection o-tile (4 heads)
HOT = 512  # out projection ho-tile


def _build_graph():
    import concourse.mybir as mybir
    import concourse.tile as tile
    from concourse import bacc

    f32 = mybir.dt.float32
    bf16 = mybir.dt.bfloat16
    AF = mybir.ActivationFunctionType

    nc = bacc.Bacc(None, target_bir_lowering=False)

    xT_d = nc.declare_dram_parameter("xT", [HID, CH], bf16, isOutput=False)
    wqT_d = nc.declare_dram_parameter("wqT", [HID, QKV_OUT], bf16, isOutput=False)
    woT_d = nc.declare_dram_parameter("woT", [HID, HID], bf16, isOutput=False)
    ctt_d = nc.declare_dram_parameter("ctt", [CH, HD], bf16, isOutput=False)
    stt_d = nc.declare_dram_parameter("stt", [CH, HD], bf16, isOutput=False)
    out_d = nc.declare_dram_parameter("out", [CH, HID], f32, isOutput=True)

    with tile.TileContext(nc) as tc:
        with nc.allow_low_precision(reason="bf16 staging for matmul operands"):
            _body(tc, mybir, f32, bf16, AF, xT_d, wqT_d, woT_d, ctt_d, stt_d, out_d)
    nc.finalize()
    return nc


class _Ctx:
    pass


def _body(tc, mybir, f32, bf16, AF, xT_d, wqT_d, woT_d, ctt_d, stt_d, out_d):
    from concourse.masks import make_identity

    nc = tc.nc
    c = _Ctx()
    c.nc = nc
    c.mybir = mybir
    c.f32, c.bf16, c.AF = f32, bf16, AF

    with tc.tile_pool(name="consts", bufs=1) as consts:
        c.bias_eps = consts.tile([128, 1], f32)
        nc.vector.memset(c.bias_eps[:], float(HD) * EPS)
        c.bias_shift = consts.tile([128, 1], f32)
        nc.vector.memset(c.bias_shift[:], ESHIFT)
        c.ident = consts.tile([128, 128], bf16)
        make_identity(nc, c.ident[:])
        c.ones = consts.tile([128, 128], bf16)
        nc.vector.memset(c.ones[:], 1.0)

        qkvp = tc.alloc_tile_pool(name="qkvp", bufs=1)
        c.ctt = qkvp.tile([128, TT, HD], bf16)
        c.stt = qkvp.tile([128, TT, HD], bf16)
        nc.sync.dma_start(c.ctt[:], ctt_d.rearrange("(a p) d -> p a d", p=128))
        nc.sync.dma_start(c.stt[:], stt_d.rearrange("(a p) d -> p a d", p=128))

        c.qT = qkvp.tile([128, NHQ, CH], bf16)
        c.kT = qkvp.tile([128, NHKV, CH], bf16)
        c.v = qkvp.tile([128, NHKV * TT, HD], bf16)

        _phase_qkv(tc, c, xT_d, wqT_d)

        oT_pool = tc.alloc_tile_pool(name="oTp", bufs=1, side="right")
        c.oTT = oT_pool.tile([128, NHQ, CH], bf16)
        _phase_attention(tc, c)
        qkvp.release()
        _phase_out_proj(tc, c, woT_d, out_d)
        oT_pool.release()


def _phase_qkv(tc, c, xT_d, wqT_d):
    nc = c.nc
    f32, bf16 = c.f32, c.bf16

    with (
        tc.tile_pool(name="xT", bufs=1) as xTp,
        tc.tile_pool(name="wt", bufs=2) as wtp,
        tc.tile_pool(name="psA", bufs=5, space="PSUM") as psA,
        tc.tile_pool(name="psT", bufs=3, space="PSUM") as psTp,
        tc.tile_pool(name="scr", bufs=3) as scr,
        tc.tile_pool(name="stats", bufs=6) as stats,
        tc.tile_pool(name="qstg", bufs=4) as qstgp,
    ):
        xTv = xT_d.rearrange("(k p) t -> p k t", p=128)
        xTall = xTp.tile([128, KC, CH], bf16)
        wqv = wqT_d.rearrange("(k p) o -> p k o", p=128)

        def load_wt(ot, nsplit=1):
            wt = wtp.tile([128, KC, OT], bf16, tag="wt", name="wt")
            step = KC // nsplit
            for s in range(nsplit):
                nc.sync.dma_start(
                    wt[:, s * step : (s + 1) * step, :],
                    wqv[:, s * step : (s + 1) * step, ot * OT : (ot + 1) * OT],
                )
            return wt

        # truly interleave xT chunk loads with the first wt group's per-chunk
        # sub-loads so the k=0 operands of both sides arrive first
        wt_next = wtp.tile([128, KC, OT], bf16, tag="wt", name="wt0")
        for k in range(KC):
            nc.sync.dma_start(wt_next[:, k, :], wqv[:, k, 0:OT])
            nc.sync.dma_start(xTall[:, k, :], xTv[:, k, :])

        pending = []  # deferred PE transposes (1 psum-tile deep pipeline)

        def flush_pending():
            while pending:
                pending.pop(0)()

        n_ot = QKV_OUT // OT  # 10
        for ot in range(n_ot):
            o0 = ot * OT
            wt = wt_next
            if ot + 1 < n_ot:
                wt_next = load_wt(ot + 1, nsplit=4)
            for t in range(TT):
                ps = psA.tile([128, OT], f32, tag="psA", name="psA")
                for k in range(KC):
                    nc.tensor.matmul(
                        ps[:],
                        lhsT=xTall[:, k, t * 128 : (t + 1) * 128],
                        rhs=wt[:, k, :],
                        start=(k == 0),
                        stop=(k == KC - 1),
                    )
                flush_pending()
                for half in range(OT // 256):
                    _evict_qkv_pair(
                        c, ps[:, half * 256 : (half + 1) * 256], o0 + half * 256,
                        t, scr, stats, qstgp, psTp, pending,
                    )
        flush_pending()


def _evict_qkv_pair(c, ps, o0, t, scr, stats, qstgp, psTp, pending):
    """Consume a [128, 256] fp32 qkv PSUM slice (2 heads)."""
    nc = c.nc
    f32, bf16, AF = c.f32, c.bf16, c.AF

    if o0 >= (NHQ + NHKV) * HD:  # v region: plain bf16 cast, natural layout
        for j in range(2):
            vh = (o0 - (NHQ + NHKV) * HD) // HD + j
            nc.scalar.copy(c.v[:, vh * TT + t, :], ps[:, j * HD : (j + 1) * HD])
        return

    if o0 < NHQ * HD:
        dstT, h0 = c.qT, o0 // HD
    else:
        dstT, h0 = c.kT, (o0 - NHQ * HD) // HD

    # RMS stats: per-head sum of squares via ACT accumulate
    sq = scr.tile([128, HD], f32, tag="sq", name="sq")
    ssq2 = stats.tile([128, 2], f32, tag="ssq", name="ssq2")
    for j in range(2):
        nc.scalar.activation(
            sq[:], ps[:, j * HD : (j + 1) * HD], AF.Square,
            accum_out=ssq2[:, j : j + 1],
        )
    rt2 = stats.tile([128, 2], f32, tag="rt", name="rt2")
    nc.scalar.activation(rt2[:], ssq2[:], AF.Sqrt, bias=c.bias_eps[:], scale=1.0)
    rr2 = stats.tile([128, 2], f32, tag="rr", name="rr2")
    nc.vector.reciprocal(rr2[:], rt2[:])

    # qn = q / rms in (half, head, d) permuted bf16 layout: RoPE ops below are
    # contiguous 2D [128, 128] covering both heads in the DVE 4x bf16 mode
    qn = scr.tile([128, 256], bf16, tag="qn", name="qn")
    nc.vector.tensor_mul(
        qn.rearrange("p (f h d) -> p f h d", f=2, h=2),
        ps.rearrange("p (h f d) -> p f h d", h=2, f=2),
        rr2.rearrange("p h -> p () h ()").to_broadcast((128, 2, 2, HH)),
    )

    ct = c.ctt[:, t, :]  # [ct | ct] matches the (h0, h1) lo/hi block layout
    st = c.stt[:, t, :]
    qs = qstgp.tile([128, 256], bf16, tag="qs", name="qs")
    qs_h = qs.rearrange("p (h f d) -> p h f d", h=2, f=2)
    t0 = scr.tile([128, HD], bf16, tag="t0", name="t0")
    t1 = scr.tile([128, HD], bf16, tag="t1", name="t1")
    nc.vector.tensor_mul(t0[:], qn[:, 0:HD], ct)
    nc.vector.tensor_mul(t1[:], qn[:, HD:256], st)
    nc.vector.tensor_sub(
        qs_h[:, :, 0, :],
        t0.rearrange("p (h d) -> p h d", h=2),
        t1.rearrange("p (h d) -> p h d", h=2),
    )
    t2 = scr.tile([128, HD], bf16, tag="t0", name="t2")
    t3 = scr.tile([128, HD], bf16, tag="t1", name="t3")
    nc.vector.tensor_mul(t2[:], qn[:, HD:256], ct)
    nc.vector.tensor_mul(t3[:], qn[:, 0:HD], st)
    nc.vector.tensor_add(
        qs_h[:, :, 1, :],
        t2.rearrange("p (h d) -> p h d", h=2),
        t3.rearrange("p (h d) -> p h d", h=2),
    )

    is_q = o0 < NHQ * HD

    def emit_transposes(qs=qs, dstT=dstT, h0=h0, t=t, is_q=is_q):
        for j in range(2):
            pst = psTp.tile([128, 128], bf16, tag="psT", name="psT")
            nc.tensor.transpose(pst[:], qs[:, j * HD : (j + 1) * HD], c.ident[:])
            # split the PSUM->SBUF evictions between DVE and ACT to balance
            if (t + j) % 2 == 0:
                nc.vector.tensor_copy(dstT[:, h0 + j, t * 128 : (t + 1) * 128], pst[:])
            else:
                nc.scalar.copy(dstT[:, h0 + j, t * 128 : (t + 1) * 128], pst[:])

    pending.append(emit_transposes)


def _phase_attention(tc, c):
    nc = c.nc
    f32, bf16, AF = c.f32, c.bf16, c.AF
    QC = 512
    NQC = CH // QC  # 2

    with (
        tc.tile_pool(name="Pp", bufs=4) as Pp,
        tc.tile_pool(name="psS", bufs=2, space="PSUM") as psS,
        tc.tile_pool(name="psAV", bufs=2, space="PSUM") as psAV,
        tc.tile_pool(name="psD", bufs=2, space="PSUM") as psD,
        tc.tile_pool(name="rsb", bufs=3) as rsbp,
        tc.tile_pool(name="ptree", bufs=2) as ptree,
    ):
        def emit_item(cur, prev):
            """Emit scores+exp+tree for `cur`, interleaving the av matmuls of
            `prev` between the score blocks so the PE always has queued work
            while ACT drains the exps."""
            h, qc = cur
            g = h // GQ
            Pt = Pp.tile([128, TT, QC], bf16, tag="P", name="Pt")
            s = []
            if prev is not None:
                ph, pqc, pPt, ps_tree = prev
                pg = ph // GQ
                pav = psAV.tile([128, QC], f32, tag="psAV", name="pav")
            for kc2 in range(TT // 2):
                pss = psS.tile([128, 2 * QC], f32, tag="psS", name="psS")
                for j in range(2):
                    nc.tensor.matmul(
                        pss[:, j * QC : (j + 1) * QC],
                        lhsT=c.kT[:, g, (2 * kc2 + j) * 128 : (2 * kc2 + j + 1) * 128],
                        rhs=c.qT[:, h, qc * QC : (qc + 1) * QC],
                        start=True,
                        stop=True,
                    )
                if prev is not None:  # 2 av matmuls of prev between S blocks
                    for j in range(2):
                        kc = 2 * kc2 + j
                        nc.tensor.matmul(
                            pav[:],
                            lhsT=c.v[:, pg * TT + kc, :],
                            rhs=pPt[:, kc, :],
                            start=(kc == 0),
                            stop=(kc == TT - 1),
                        )
                nc.scalar.activation(
                    Pt.rearrange("p a b -> p (a b)")[
                        :, 2 * kc2 * QC : 2 * (kc2 + 1) * QC
                    ],
                    pss[:], AF.Exp, bias=c.bias_shift[:], scale=ESCALE,
                )
                sj = ptree.tile([128, QC], bf16, tag=f"pt{kc2}", name=f"s{kc2}")
                nc.vector.tensor_add(
                    sj[:], Pt[:, 2 * kc2, :], Pt[:, 2 * kc2 + 1, :]
                )
                s.append(sj)
            if prev is not None:
                pd = psD.tile([128, QC], f32, tag="psD", name="pd")
                for j in range(4):
                    nc.tensor.matmul(
                        pd[:], lhsT=c.ones[:], rhs=ps_tree[j][:],
                        start=(j == 0), stop=(j == 3),
                    )
                rsb = rsbp.tile([128, QC], f32, tag="rsb", name="rsb")
                nc.vector.reciprocal(rsb[:], pd[:])
                nc.vector.tensor_mul(
                    c.oTT[:, ph, pqc * QC : (pqc + 1) * QC], pav[:], rsb[:]
                )
            return Pt, s

        def emit_tail(prev):
            ph, pqc, pPt, ps_tree = prev
            pg = ph // GQ
            pav = psAV.tile([128, QC], f32, tag="psAV", name="pav")
            for kc in range(TT):
                nc.tensor.matmul(
                    pav[:],
                    lhsT=c.v[:, pg * TT + kc, :],
                    rhs=pPt[:, kc, :],
                    start=(kc == 0),
                    stop=(kc == TT - 1),
                )
            pd = psD.tile([128, QC], f32, tag="psD", name="pd")
            for j in range(4):
                nc.tensor.matmul(
                    pd[:], lhsT=c.ones[:], rhs=ps_tree[j][:],
                    start=(j == 0), stop=(j == 3),
                )
            rsb = rsbp.tile([128, QC], f32, tag="rsb", name="rsb")
            nc.vector.reciprocal(rsb[:], pd[:])
            nc.vector.tensor_mul(
                c.oTT[:, ph, pqc * QC : (pqc + 1) * QC], pav[:], rsb[:]
            )

        work = [(h, qc) for h in range(NHQ) for qc in range(NQC)]
        prev = None
        for cur in work:
            Pt, s = emit_item(cur, prev)
            prev = (cur[0], cur[1], Pt, s)
        emit_tail(prev)


def _phase_out_proj(tc, c, woT_d, out_d):
    nc = c.nc
    f32, bf16 = c.f32, c.bf16

    with (
        tc.tile_pool(name="wt2", bufs=2) as wtp,
        tc.tile_pool(name="psB", bufs=4, space="PSUM") as psB,
        tc.tile_pool(name="outs", bufs=3) as outs,
    ):
        wov = woT_d.rearrange("(k p) o -> p k o", p=128)

        def load_wt2(ho):
            wt = wtp.tile([128, KC, HOT], bf16, tag="wt2", name="wt2")
            nc.sync.dma_start(wt[:], wov[:, :, ho * HOT : (ho + 1) * HOT])
            return wt

        n_ho = HID // HOT  # 6
        wt_next = load_wt2(0)
        for ho in range(n_ho):
            ho0 = ho * HOT
            wt = wt_next
            if ho + 1 < n_ho:
                wt_next = load_wt2(ho + 1)
            for t in range(TT):
                ps = psB.tile([128, HOT], f32, tag="psB", name="psB")
                for k in range(KC):
                    nc.tensor.matmul(
                        ps[:],
                        lhsT=c.oTT[:, k, t * 128 : (t + 1) * 128],
                        rhs=wt[:, k, :],
                        start=(k == 0),
                        stop=(k == KC - 1),
                    )
                ob = outs.tile([128, HOT], f32, tag="outs", name="ob")
                nc.scalar.copy(ob[:], ps[:])
                nc.gpsimd.dma_start(
                    out_d[t * 128 : (t + 1) * 128, ho0 : ho0 + HOT], ob[:]
                )


_NC_CACHE = None


def _get_nc():
    global _NC_CACHE
    if _NC_CACHE is None:
        _NC_CACHE = _build_graph()
    return _NC_CACHE


def kernel(**inputs) -> np.ndarray:
    import ml_dtypes

    from concourse.bass_utils import run_bass_kernel_spmd

    bf16 = ml_dtypes.bfloat16
    x = np.asarray(inputs["x"], dtype=np.float32)
    w_qkv = np.asarray(inputs["w_qkv"], dtype=np.float32)
    w_out = np.asarray(inputs["w_out"], dtype=np.float32)
    cos = np.asarray(inputs["cos"], dtype=np.float32)
    sin = np.asarray(inputs["sin"], dtype=np.float32)

    # host-side marshalling: per-modality weight transposes (shared by the 4
    # cores of each modality), bf16 compute dtype, rotate-half cos/sin layout
    wqT = [np.ascontiguousarray(w_qkv[m].T).astype(bf16) for m in range(NM)]
    woT = [np.ascontiguousarray(w_out[m].T).astype(bf16) for m in range(NM)]

    in_maps = []
    for i in range(NCORES):
        m = i * NM // NCORES  # cores 0-3 -> modality 0, 4-7 -> modality 1
        sl = slice(i * CH, (i + 1) * CH)
        ctt = np.concatenate([cos[sl], cos[sl]], axis=1).astype(bf16)
        stt = np.concatenate([sin[sl], sin[sl]], axis=1).astype(bf16)
        in_maps.append(
            {
                "xT": np.ascontiguousarray(x[sl].T).astype(bf16),
                "wqT": wqT[m],
                "woT": woT[m],
                "ctt": ctt,
                "stt": stt,
            }
        )

    nc = _get_nc()
    res = run_bass_kernel_spmd(nc, in_maps, core_ids=list(range(NCORES)))
    outs = [np.asarray(res.results[i]["out"]) for i in range(NCORES)]
    return np.concatenate(outs, axis=0).astype(np.float32)


# revision 39
# speedup vs baseline: 1.1407x; 1.0022x over previous
"""Trainium2 Bass kernel for nn_Attention_29635274342682 (sparse_attention).

Reference semantics: per-modality (MoE) QKV projection -> per-head RMS-norm
(weight zeros -> scale 1) -> RoPE -> block-diagonal attention over 8 chunks
of 1024 tokens (GQA 24q/8kv heads, hd=128) -> per-modality output projection.
Biases / norm weights are zeros by construction (spec fill "zeros"), so they
are not device inputs.

Sharding: context parallel, core i <- token chunk i (1024 tokens).  Chunk
boundaries coincide with both the attention ranges (CHUNK=1024) and the
modality split (4 chunks per modality), so there is NO cross-core
communication: each core runs the full pipeline on its chunk with its
modality's weights.

Host-side marshalling (in kernel(), pure layout work, no FLOPs): inputs are
sliced per core, cast to bf16 (matmul compute dtype; fp32 accumulation on
device) and pre-transposed so every matmul operand arrives contraction-on-
partitions via plain strided DMA loads.  cos/sin are pre-duplicated to the
rotate-half layout.

Device pipeline per core:
  1. qkv[t,o] = xT.T @ wqT       (PSUM fp32, o-tiles of 512 = 4 heads)
  2. q/k: RMS norm over head dim + RoPE, batched 2 heads per DVE op in the
     bf16 4x mode; the 1/HD mean factor of the RMS norm is folded into the
     softmax exp scale.  bf16 staging is transposed to qT/kT [hd, t] on the
     PE (identity transpose), software-pipelined one psum-tile behind.
  3. scoresT[kt, qt] = kT.T @ qT; P = exp(s*scale - sqrt(HD)) on ACT
     (shift is softmax-invariant; Cauchy-Schwarz bounds |s| <= sqrt(HD)).
  4. oT_unnorm[hd, q] = v.T-chunks @ P-chunks (N=512); the softmax
     denominator is computed partition-broadcast by all-ones stationary
     matmuls over pairwise k-chunk sums of P (DVE add-tree splits the
     reduction between DVE and PE); DVE reciprocal + multiply -> oT bf16.
  5. out[t, ho] = oT.T @ woT -> fp32 -> DRAM.

Measured on HW (trn2, 8 cores): ~0.99 ms NEFF exec, rel err ~6.8e-3 vs the
fp32 reference.  The PE runs saturated back-to-back N=512 bf16 matmuls
(~216 ns issue rate) through all three phases.
"""

import os
import sys

import numpy as np

if os.path.isdir("/opt/trn_rl_repo") and "/opt/trn_rl_repo" not in sys.path:
    sys.path.insert(0, "/opt/trn_rl_repo")

S = 8192
HID = 3072
NHQ = 24
NHKV = 8
GQ = NHQ // NHKV  # 3
HD = 128
HH = HD // 2
NM = 2
CH = 1024  # tokens per core == attention chunk
QKV_OUT = (NHQ + 2 * NHKV) * HD  # 5120
EPS = 1e-6
NCORES = 8
TT = CH // 128  # 8 token tiles per core
KC = HID // 128  # 24 contraction chunks

ESCALE = float(HD) ** 0.5
ESHIFT = -(float(HD) ** 0.5)

OT = 512  # qkv projection o-tile (4 heads)
HOT = 512  # out projection ho-tile


def _build_graph():
    import concourse.mybir as mybir
    import concourse.tile as tile
    from concourse import bacc

    f32 = mybir.dt.float32
    bf16 = mybir.dt.bfloat16
    AF = mybir.ActivationFunctionType

    nc = bacc.Bacc(None, target_bir_lowering=False)

    xT_d = nc.declare_dram_parameter("xT", [HID, CH], bf16, isOutput=False)
    wqT_d = nc.declare_dram_parameter("wqT", [HID, QKV_OUT], bf16, isOutput=False)
    woT_d = nc.declare_dram_parameter("woT", [HID, HID], bf16, isOutput=False)
    ctt_d = nc.declare_dram_parameter("ctt", [CH, HD], bf16, isOutput=False)
    stt_d = nc.declare_dram_parameter("stt", [CH, HD], bf16, isOutput=False)
    out_d = nc.declare_dram_parameter("out", [CH, HID], f32, isOutput=True)

    with tile.TileContext(nc) as tc:
        with nc.allow_low_precision(reason="bf16 staging for matmul operands"):
            _body(tc, mybir, f32, bf16, AF, xT_d, wqT_d, woT_d, ctt_d, stt_d, out_d)
    nc.finalize()
    return nc


class _Ctx:
    pass


def _body(tc, mybir, f32, bf16, AF, xT_d, wqT_d, woT_d, ctt_d, stt_d, out_d):
    from concourse.masks import make_identity

    nc = tc.nc
    c = _Ctx()
    c.nc = nc
    c.mybir = mybir
    c.f32, c.bf16, c.AF = f32, bf16, AF

    with tc.tile_pool(name="consts", bufs=1) as consts:
        c.bias_eps = consts.tile([128, 1], f32)
        nc.vector.memset(c.bias_eps[:], float(HD) * EPS)
        c.bias_shift = consts.tile([128, 1], f32)
        nc.vector.memset(c.bias_shift[:], ESHIFT)
        c.ident = consts.tile([128, 128], bf16)
        make_identity(nc, c.ident[:])
        c.ones = consts.tile([128, 128], bf16)
        nc.vector.memset(c.ones[:], 1.0)

        qkvp = tc.alloc_tile_pool(name="qkvp", bufs=1)
        c.ctt = qkvp.tile([128, TT, HD], bf16)
        c.stt = qkvp.tile([128, TT, HD], bf16)
        nc.sync.dma_start(c.ctt[:], ctt_d.rearrange("(a p) d -> p a d", p=128))
        nc.sync.dma_start(c.stt[:], stt_d.rearrange("(a p) d -> p a d", p=128))

        c.qT = qkvp.tile([128, NHQ, CH], bf16)
        c.kT = qkvp.tile([128, NHKV, CH], bf16)
        c.v = qkvp.tile([128, NHKV * TT, HD], bf16)

        _phase_qkv(tc, c, xT_d, wqT_d)

        oT_pool = tc.alloc_tile_pool(name="oTp", bufs=1, side="right")
        c.oTT = oT_pool.tile([128, NHQ, CH], bf16)
        _phase_attention(tc, c)
        qkvp.release()
        _phase_out_proj(tc, c, woT_d, out_d)
        oT_pool.release()


def _phase_qkv(tc, c, xT_d, wqT_d):
    nc = c.nc
    f32, bf16 = c.f32, c.bf16

    with (
        tc.tile_pool(name="xT", bufs=1) as xTp,
        tc.tile_pool(name="wt", bufs=2) as wtp,
        tc.tile_pool(name="psA", bufs=5, space="PSUM") as psA,
        tc.tile_pool(name="psT", bufs=3, space="PSUM") as psTp,
        tc.tile_pool(name="scr", bufs=3) as scr,
        tc.tile_pool(name="stats", bufs=6) as stats,
        tc.tile_pool(name="qstg", bufs=4) as qstgp,
    ):
        xTv = xT_d.rearrange("(k p) t -> p k t", p=128)
        xTall = xTp.tile([128, KC, CH], bf16)
        wqv = wqT_d.rearrange("(k p) o -> p k o", p=128)

        def load_wt(ot, nsplit=1):
            wt = wtp.tile([128, KC, OT], bf16, tag="wt", name="wt")
            step = KC // nsplit
            for s in range(nsplit):
                nc.sync.dma_start(
                    wt[:, s * step : (s + 1) * step, :],
                    wqv[:, s * step : (s + 1) * step, ot * OT : (ot + 1) * OT],
                )
            return wt

        # truly interleave xT chunk loads with the first wt group's per-chunk
        # sub-loads so the k=0 operands of both sides arrive first
        wt_next = wtp.tile([128, KC, OT], bf16, tag="wt", name="wt0")
        for k in range(KC):
            nc.sync.dma_start(wt_next[:, k, :], wqv[:, k, 0:OT])
            nc.sync.dma_start(xTall[:, k, :], xTv[:, k, :])

        pending = []  # deferred PE transposes (1 psum-tile deep pipeline)

        def flush_pending():
            while pending:
                pending.pop(0)()

        n_ot = QKV_OUT // OT  # 10
        for ot in range(n_ot):
            o0 = ot * OT
            wt = wt_next
            if ot + 1 < n_ot:
                wt_next = load_wt(ot + 1, nsplit=4)
            for t in range(TT):
                ps = psA.tile([128, OT], f32, tag="psA", name="psA")
                for k in range(KC):
                    nc.tensor.matmul(
                        ps[:],
                        lhsT=xTall[:, k, t * 128 : (t + 1) * 128],
                        rhs=wt[:, k, :],
                        start=(k == 0),
                        stop=(k == KC - 1),
                    )
                flush_pending()
                for half in range(OT // 256):
                    _evict_qkv_pair(
                        c, ps[:, half * 256 : (half + 1) * 256], o0 + half * 256,
                        t, scr, stats, qstgp, psTp, pending,
                    )
        flush_pending()


def _evict_qkv_pair(c, ps, o0, t, scr, stats, qstgp, psTp, pending):
    """Consume a [128, 256] fp32 qkv PSUM slice (2 heads)."""
    nc = c.nc
    f32, bf16, AF = c.f32, c.bf16, c.AF

    if o0 >= (NHQ + NHKV) * HD:  # v region: plain bf16 cast, natural layout
        for j in range(2):
            vh = (o0 - (NHQ + NHKV) * HD) // HD + j
            nc.scalar.copy(c.v[:, vh * TT + t, :], ps[:, j * HD : (j + 1) * HD])
        return

    if o0 < NHQ * HD:
        dstT, h0 = c.qT, o0 // HD
    else:
        dstT, h0 = c.kT, (o0 - NHQ * HD) // HD

    # RMS stats: per-head sum of squares via ACT accumulate
    sq = scr.tile([128, HD], f32, tag="sq", name="sq")
    ssq2 = stats.tile([128, 2], f32, tag="ssq", name="ssq2")
    for j in range(2):
        nc.scalar.activation(
            sq[:], ps[:, j * HD : (j + 1) * HD], AF.Square,
            accum_out=ssq2[:, j : j + 1],
        )
    rt2 = stats.tile([128, 2], f32, tag="rt", name="rt2")
    nc.scalar.activation(rt2[:], ssq2[:], AF.Sqrt, bias=c.bias_eps[:], scale=1.0)
    rr2 = stats.tile([128, 2], f32, tag="rr", name="rr2")
    nc.vector.reciprocal(rr2[:], rt2[:])

    # qn = q / rms in (half, head, d) permuted bf16 layout: RoPE ops below are
    # contiguous 2D [128, 128] covering both heads in the DVE 4x bf16 mode
    qn = scr.tile([128, 256], bf16, tag="qn", name="qn")
    nc.vector.tensor_mul(
        qn.rearrange("p (f h d) -> p f h d", f=2, h=2),
        ps.rearrange("p (h f d) -> p f h d", h=2, f=2),
        rr2.rearrange("p h -> p () h ()").to_broadcast((128, 2, 2, HH)),
    )

    ct = c.ctt[:, t, :]  # [ct | ct] matches the (h0, h1) lo/hi block layout
    st = c.stt[:, t, :]
    qs = qstgp.tile([128, 256], bf16, tag="qs", name="qs")
    qs_h = qs.rearrange("p (h f d) -> p h f d", h=2, f=2)
    t0 = scr.tile([128, HD], bf16, tag="t0", name="t0")
    t1 = scr.tile([128, HD], bf16, tag="t1", name="t1")
    nc.vector.tensor_mul(t0[:], qn[:, 0:HD], ct)
    nc.vector.tensor_mul(t1[:], qn[:, HD:256], st)
    nc.vector.tensor_sub(
        qs_h[:, :, 0, :],
        t0.rearrange("p (h d) -> p h d", h=2),
        t1.rearrange("p (h d) -> p h d", h=2),
    )
    t2 = scr.tile([128, HD], bf16, tag="t0", name="t2")
    t3 = scr.tile([128, HD], bf16, tag="t1", name="t3")
    nc.vector.tensor_mul(t2[:], qn[:, HD:256], ct)
    nc.vector.tensor_mul(t3[:], qn[:, 0:HD], st)
    nc.vector.tensor_add(
        qs_h[:, :, 1, :],
        t2.rearrange("p (h d) -> p h d", h=2),
        t3.rearrange("p (h d) -> p h d", h=2),
    )

    is_q = o0 < NHQ * HD

    def emit_transposes(qs=qs, dstT=dstT, h0=h0, t=t, is_q=is_q):
        for j in range(2):
            pst = psTp.tile([128, 128], bf16, tag="psT", name="psT")
            nc.tensor.transpose(pst[:], qs[:, j * HD : (j + 1) * HD], c.ident[:])
            # split the PSUM->SBUF evictions between DVE and ACT to balance
            if (t + j) % 2 == 0:
                nc.vector.tensor_copy(dstT[:, h0 + j, t * 128 : (t + 1) * 128], pst[:])
            else:
                nc.scalar.copy(dstT[:, h0 + j, t * 128 : (t + 1) * 128], pst[:])

    pending.append(emit_transposes)


def _phase_attention(tc, c):
    nc = c.nc
    f32, bf16, AF = c.f32, c.bf16, c.AF
    QC = 512
    NQC = CH // QC  # 2

    with (
        tc.tile_pool(name="Pp", bufs=4) as Pp,
        tc.tile_pool(name="psS", bufs=2, space="PSUM") as psS,
        tc.tile_pool(name="psAV", bufs=2, space="PSUM") as psAV,
        tc.tile_pool(name="psD", bufs=2, space="PSUM") as psD,
        tc.tile_pool(name="rsb", bufs=3) as rsbp,
        tc.tile_pool(name="ptree", bufs=2) as ptree,
    ):
        def emit_item(cur, prev):
            """Emit scores+exp+tree for `cur`, interleaving the av matmuls of
            `prev` between the score blocks so the PE always has queued work
            while ACT drains the exps."""
            h, qc = cur
            g = h // GQ
            Pt = Pp.tile([128, TT, QC], bf16, tag="P", name="Pt")
            s = []
            if prev is not None:
                ph, pqc, pPt, ps_tree = prev
                pg = ph // GQ
                pav = psAV.tile([128, QC], f32, tag="psAV", name="pav")
            if prev is not None:
                pd = psD.tile([128, QC], f32, tag="psD", name="pd")
            for kc2 in range(TT // 2):
                pss = psS.tile([128, 2 * QC], f32, tag="psS", name="psS")
                for j in range(2):
                    nc.tensor.matmul(
                        pss[:, j * QC : (j + 1) * QC],
                        lhsT=c.kT[:, g, (2 * kc2 + j) * 128 : (2 * kc2 + j + 1) * 128],
                        rhs=c.qT[:, h, qc * QC : (qc + 1) * QC],
                        start=True,
                        stop=True,
                    )
                if prev is not None:
                    # 2 av matmuls of prev between S blocks; the den matmuls
                    # ride in the last two blocks so only 2 filler matmuls
                    # separate S3 from the next item's S0 (keeps ACT fed
                    # across the item boundary)
                    for j in range(2):
                        kc = 2 * kc2 + j
                        nc.tensor.matmul(
                            pav[:],
                            lhsT=c.v[:, pg * TT + kc, :],
                            rhs=pPt[:, kc, :],
                            start=(kc == 0),
                            stop=(kc == TT - 1),
                        )
                    if kc2 >= 2:
                        for j in range(2):
                            dj = 2 * (kc2 - 2) + j
                            nc.tensor.matmul(
                                pd[:], lhsT=c.ones[:], rhs=ps_tree[dj][:],
                                start=(dj == 0), stop=(dj == 3),
                            )
                nc.scalar.activation(
                    Pt.rearrange("p a b -> p (a b)")[
                        :, 2 * kc2 * QC : 2 * (kc2 + 1) * QC
                    ],
                    pss[:], AF.Exp, bias=c.bias_shift[:], scale=ESCALE,
                )
                sj = ptree.tile([128, QC], bf16, tag=f"pt{kc2}", name=f"s{kc2}")
                nc.vector.tensor_add(
                    sj[:], Pt[:, 2 * kc2, :], Pt[:, 2 * kc2 + 1, :]
                )
                s.append(sj)
            if prev is not None:
                rsb = rsbp.tile([128, QC], f32, tag="rsb", name="rsb")
                nc.vector.reciprocal(rsb[:], pd[:])
                nc.vector.tensor_mul(
                    c.oTT[:, ph, pqc * QC : (pqc + 1) * QC], pav[:], rsb[:]
                )
            return Pt, s

        def emit_tail(prev):
            ph, pqc, pPt, ps_tree = prev
            pg = ph // GQ
            pav = psAV.tile([128, QC], f32, tag="psAV", name="pav")
            for kc in range(TT):
                nc.tensor.matmul(
                    pav[:],
                    lhsT=c.v[:, pg * TT + kc, :],
                    rhs=pPt[:, kc, :],
                    start=(kc == 0),
                    stop=(kc == TT - 1),
                )
            pd = psD.tile([128, QC], f32, tag="psD", name="pd")
            for j in range(4):
                nc.tensor.matmul(
                    pd[:], lhsT=c.ones[:], rhs=ps_tree[j][:],
                    start=(j == 0), stop=(j == 3),
                )
            rsb = rsbp.tile([128, QC], f32, tag="rsb", name="rsb")
            nc.vector.reciprocal(rsb[:], pd[:])
            nc.vector.tensor_mul(
                c.oTT[:, ph, pqc * QC : (pqc + 1) * QC], pav[:], rsb[:]
            )

        work = [(h, qc) for h in range(NHQ) for qc in range(NQC)]
        prev = None
        for cur in work:
            Pt, s = emit_item(cur, prev)
            prev = (cur[0], cur[1], Pt, s)
        emit_tail(prev)


def _phase_out_proj(tc, c, woT_d, out_d):
    nc = c.nc
    f32, bf16 = c.f32, c.bf16

    with (
        tc.tile_pool(name="wt2", bufs=2) as wtp,
        tc.tile_pool(name="psB", bufs=4, space="PSUM") as psB,
        tc.tile_pool(name="outs", bufs=3) as outs,
    ):
        wov = woT_d.rearrange("(k p) o -> p k o", p=128)

        def load_wt2(ho):
            wt = wtp.tile([128, KC, HOT], bf16, tag="wt2", name="wt2")
            nc.sync.dma_start(wt[:], wov[:, :, ho * HOT : (ho + 1) * HOT])
            return wt

        n_ho = HID // HOT  # 6
        wt_next = load_wt2(0)
        for ho in range(n_ho):
            ho0 = ho * HOT
            wt = wt_next
            if ho + 1 < n_ho:
                wt_next = load_wt2(ho + 1)
            for t in range(TT):
                ps = psB.tile([128, HOT], f32, tag="psB", name="psB")
                for k in range(KC):
                    nc.tensor.matmul(
                        ps[:],
                        lhsT=c.oTT[:, k, t * 128 : (t + 1) * 128],
                        rhs=wt[:, k, :],
                        start=(k == 0),
                        stop=(k == KC - 1),
                    )
                ob = outs.tile([128, HOT], f32, tag="outs", name="ob")
                nc.scalar.copy(ob[:], ps[:])
                nc.gpsimd.dma_start(
                    out_d[t * 128 : (t + 1) * 128, ho0 : ho0 + HOT], ob[:]
                )


_NC_CACHE = None


def _get_nc():
    global _NC_CACHE
    if _NC_CACHE is None:
        _NC_CACHE = _build_graph()
    return _NC_CACHE


def kernel(**inputs) -> np.ndarray:
    import ml_dtypes

    from concourse.bass_utils import run_bass_kernel_spmd

    bf16 = ml_dtypes.bfloat16
    x = np.asarray(inputs["x"], dtype=np.float32)
    w_qkv = np.asarray(inputs["w_qkv"], dtype=np.float32)
    w_out = np.asarray(inputs["w_out"], dtype=np.float32)
    cos = np.asarray(inputs["cos"], dtype=np.float32)
    sin = np.asarray(inputs["sin"], dtype=np.float32)

    # host-side marshalling: per-modality weight transposes (shared by the 4
    # cores of each modality), bf16 compute dtype, rotate-half cos/sin layout
    wqT = [np.ascontiguousarray(w_qkv[m].T).astype(bf16) for m in range(NM)]
    woT = [np.ascontiguousarray(w_out[m].T).astype(bf16) for m in range(NM)]

    in_maps = []
    for i in range(NCORES):
        m = i * NM // NCORES  # cores 0-3 -> modality 0, 4-7 -> modality 1
        sl = slice(i * CH, (i + 1) * CH)
        ctt = np.concatenate([cos[sl], cos[sl]], axis=1).astype(bf16)
        stt = np.concatenate([sin[sl], sin[sl]], axis=1).astype(bf16)
        in_maps.append(
            {
                "xT": np.ascontiguousarray(x[sl].T).astype(bf16),
                "wqT": wqT[m],
                "woT": woT[m],
                "ctt": ctt,
                "stt": stt,
            }
        )

    nc = _get_nc()
    res = run_bass_kernel_spmd(nc, in_maps, core_ids=list(range(NCORES)))
    outs = [np.asarray(res.results[i]["out"]) for i in range(NCORES)]
    return np.concatenate(outs, axis=0).astype(np.float32)
